# revision 38
# baseline (speedup 1.0000x reference)
"""Multi-head attention (B=2, N=4096, D=512, H=8) on 8 TRN2 NeuronCores.

Sharding: head-parallel (core d owns head d, both batches). v3 layout:
  - All DRAM inputs are host-packed so every device DMA is a contiguous
    read (the transposed [D, T] view produced 1KB-segmented descriptors at
    ~30GB/s/queue); one DMA per (blk, k) — issue instructions cost ~0.62us
    each on Sync and their serial rate gates the first kq production.
    First real exp ~19.6us (was 24.5).
  - JIT production: kT/qT/v tiles are produced inside the attention loop's
    PE slack, gated on their xT blocks' DMA arrival. qc0's PV quota is
    small (8) since qc0-2 carry all one-time production (~66us PE) and are
    the oversubscribed region.
  - PV matmuls run on a quota-paced lag behind the exp stream (per-qc pair
    quotas, e-pool 32 deep) so production never starves ScalarE, the
    bottleneck: 256 ACTIVATEs of [128,1024] at (N+313)/1.2GHz = 285us busy.
    The serialized-BIR postprocess drops same-engine provably-satisfied
    waits, which removed ~165ns/exp of EventSemaphore+issue overhead.
  - Output A2A split into 4 pipelined pieces (after qc3 / qc5 / qc6 / qc7)
    with token ownership striped across completion order; single
    multi-dim-AP DMAs scatter/unpack each piece (was 4 descriptors each).
    proj subtiles 0-3 run in qc6's slack, 4-5 in early qc7; only piece 3's
    A2A (~6-25us latency, size- independent, high run-to-run variance) +
    one proj subtile + postamble are exposed as tail (~26-46us).
  - qc7's softmax reciprocal runs on the then-idle ScalarE as
    exp(-ln(den/256)) (Ln+Exp share one act table set; DVE's iterative
    reciprocal is 3.3us serialized) and broadcasts via a fp16 K=1 PE
    matmul (fp32 ran LOW_HIGH dual-pass at 2.2us).
  - proj bias is added on the host in fp32 (its K=1 matmuls stole PE slots
    in the saturated tail region).
  - fp16 everywhere off-PSUM (better mantissa than bf16; same speed).
Host side packs/casts inputs, scatters the 8 output slices, adds bias.
"""

from collections import deque
from contextlib import ExitStack

import numpy as np

N_CORES = 8
B, N, D = 2, 4096, 512
H, HD = 8, 64
T = B * N              # 8192 flattened tokens
TS = T // N_CORES      # 1024 tokens output slice per core
SCALE = HD ** -0.5
KC = D // 128          # 4 contraction chunks of the model dim
NKT = N // 128         # 32 k-token tiles per batch
QC = 512               # q-chunk processed per accumulator
NQC = N // QC          # 8 q-chunks per batch

F16 = np.float16

# PV emission quota per qc, in (kt, both-batch) pair units; sums to 256.
# Lag after each qc: 16, 22, 18, 10, 6, 4, 2, 0 — smooths the JIT
# production load of qc0-2 into later windows, keeps a small lag at late
# boundaries (so acc-ring recycling behind collective-delayed norm DMAs
# never blocks the next qc's scores), drains before the tail.
PV_QUOTA = [8, 26, 36, 42, 40, 36, 36, 32]

_COMPILED = {}


def _patch_tile_drain():
    """The walrus build in this container caps sync waits at 1 per
    instruction (2 for EventSemaphore), but TileContext._drain_and_barrier
    puts every live proc's final wait on a single Drain, which fails
    codegen with 'Too many sync wait commands'. Re-emit those waits as
    individual wait_ge instructions before the drain."""
    import concourse.mybir as mybir
    import concourse.tile as tile
    from concourse.bass_types import SemaphoreHandle
    from concourse.vector_clock import ScopedClock

    if getattr(tile.TileContext, "_drain_patch_installed", False):
        return

    def _drain_and_barrier(self, tick_clock, wait_clock):
        probe = mybir.InstNoOp(name=f"drain-probe-{self.nc.next_id()}", ins=[], outs=[])
        probe.engine = mybir.EngineType.SP
        wait_clock.add_sem_waits(probe, ScopedClock({None: tick_clock.global_clock}))
        waits = probe.sync_info.on_wait if probe.sync_info is not None else []
        for w in waits:
            assert w.wait_mode == "sem-ge-imm", w
            self.nc.sync.wait_ge(SemaphoreHandle(w.ant_name, w.id), w.wait_value)
        self.nc.sync.drain()

        self.nc.all_engine_barrier()
        assert self.sems is not None
        popped = self.nc._tile_sem_poison_stack.pop()
        assert popped is self._sem_poison
        self.nc.clear_and_free_semaphores(list(self.sems.allocated().values()))
        # NOTE: Tile's teardown ends with a second all_engine_barrier, but
        # the first barrier already proves every engine is done and the
        # semaphore clear runs on a single queue that must drain before
        # the program ends — the extra rendezvous cost ~4-7us of measured
        # tail in the trace (staggered $S[2] ring waits).

    tile.TileContext._drain_and_barrier = _drain_and_barrier
    tile.TileContext._drain_patch_installed = True


def _patch_multiwait_split():
    """This walrus build rejects instructions with more than one sync wait
    ('Too many sync wait commands'), but Tile's wait assigner can emit
    several waits on one instruction. Post-process the serialized BIR:
    first DROP waits that are provably satisfied by queue order (a wait on
    a semaphore whose only updaters are synchronous compute instructions
    on the waiting instruction's own in-order engine — Tile derived the
    wait from a happens-before on that same queue, so it is a no-op at
    execution time; this removes ~1 EventSemaphore shim per exp on the
    bottleneck Scalar queue), then move any remaining excess waits onto
    single-wait EventSemaphore instructions inserted just before the
    owning instruction (same engine => executes in order)."""
    import json

    import concourse.bass as bass

    if getattr(bass.Bass, "_multiwait_patch_installed", False):
        return
    orig = bass.Bass.to_json_bytes

    # in-order completion updaters; async completers (DMACopy,
    # CollectiveCompute) and barrier ops excluded
    _SAFE_OPS = {
        "Activation", "TensorTensor", "TensorCopy", "Reciprocal",
        "Memset", "TensorReduce", "Matmult", "Ldweights",
        "TensorScalarPtr",
    }

    def to_json_bytes(self, *a, **kw):
        data = json.loads(orig(self, *a, **kw))
        n_split = 0
        for fn in data.get("functions", []):
            upd = {}
            for bb in fn.get("blocks", []):
                for inst in bb.get("instructions") or []:
                    si = inst.get("sync_info")
                    for u in (si or {}).get("on_update") or []:
                        upd.setdefault(u["id"], set()).add(
                            (inst["engine"], inst["opcode"])
                        )
            for bb in fn.get("blocks", []):
                insts = bb.get("instructions")
                if not insts:
                    continue
                out = []
                for inst in insts:
                    si = inst.get("sync_info")
                    ow = (si or {}).get("on_wait") or []
                    if ow and inst["opcode"] not in ("Drain", "EventSemaphore"):
                        kept = [
                            w for w in ow
                            if not (
                                w.get("wait_mode") == "sem-ge-imm"
                                and upd.get(w["id"])
                                and all(
                                    e == inst["engine"] and o in _SAFE_OPS
                                    for e, o in upd[w["id"]]
                                )
                            )
                        ]
                        if len(kept) != len(ow):
                            si["on_wait"] = ow = kept
                    if len(ow) > 1:
                        for i, w in enumerate(ow[:-1]):
                            out.append({
                                "debug": inst.get("debug", 0),
                                "engine": inst["engine"],
                                "ins": [],
                                "outs": [],
                                "name": f"{inst['name']}-esw{i}",
                                "opcode": "EventSemaphore",
                                "sync_info": {"on_update": [], "on_wait": [w]},
                            })
                            n_split += 1
                        si["on_wait"] = [ow[-1]]
                    out.append(inst)
                bb["instructions"] = out
        return json.dumps(data).encode()

    bass.Bass.to_json_bytes = to_json_bytes
    bass.Bass._multiwait_patch_installed = True


def _build():
    import concourse.bass as bass
    import concourse.mybir as mybir
    import concourse.tile as tile

    _patch_tile_drain()
    _patch_multiwait_split()
    dt = mybir.dt
    nc = bass.Bass(num_devices=N_CORES)

    # All inputs are HOST-PACKED so every DMA reads a contiguous DRAM span:
    # the transposed [D, T] layout produced 1KB-segmented descriptors that
    # run at ~30GB/s/queue and put the first exp at 26us.
    #   xTb[(blk*KC + k)*128 + p, b*512 + c] = x[b, blk*512 + c, k*128 + p]
    #   w*p[p, k*HD + c] = w*T[k*128 + p, c]   (ditto wpp with D cols)
    xTb_ext = nc.declare_dram_parameter(
        "xTb", [NQC * KC * 128, B * 512], dt.float16, isOutput=False
    )
    wqT_ext = nc.declare_dram_parameter(
        "wqp", [128, KC * HD], dt.float16, isOutput=False
    )
    wkT_ext = nc.declare_dram_parameter(
        "wkp", [128, KC * HD], dt.float16, isOutput=False
    )
    wvT_ext = nc.declare_dram_parameter(
        "wvp", [128, KC * HD], dt.float16, isOutput=False
    )
    wpT_ext = nc.declare_dram_parameter(
        "wpp", [128, KC * D], dt.float16, isOutput=False
    )
    out_ext = nc.declare_dram_parameter("out", [TS, D], dt.float16, isOutput=True)

    with tile.TileContext(nc) as tc, ExitStack() as ctx:
        singles = ctx.enter_context(tc.tile_pool(name="singles", bufs=1))
        dram = ctx.enter_context(tc.tile_pool(name="dram", bufs=4, space="DRAM"))
        cpool = ctx.enter_context(tc.tile_pool(name="cpool", bufs=4))

        # ---------- persistent SBUF ----------
        xT_k = [
            singles.tile([128, T], dt.float16, tag=f"xT{k}", name=f"xT{k}")
            for k in range(KC)
        ]
        wqT_sb = singles.tile([128, KC, HD], dt.float16)
        wkT_sb = singles.tile([128, KC, HD], dt.float16)
        wvT_sb = singles.tile([128, KC, HD], dt.float16)
        wpT_sb = singles.tile([128, KC, D], dt.float16)
        ones_sb = singles.tile([1, 128], dt.float16)
        ones65h = singles.tile([1, 1 + HD], dt.float16)
        # NOTE: merging these into single 3D tiles (kT/qT/vp as one tile
        # each) shifted downstream SBUF addresses and made EVERY exp
        # ACTIVATE ~223ns slower (min dur 1104->1327ns) — layout/alignment
        # sensitivity. Keep them as separate tiles.
        kT_t = [
            singles.tile([128, 512], dt.float16, tag=f"kT{i}", name=f"kT{i}")
            for i in range(NQC)
        ]
        qT_t = [
            singles.tile([128, 512], dt.float16, tag=f"qT{i}", name=f"qT{i}")
            for i in range(NQC)
        ]
        vp_t = [
            singles.tile([128, 1 + HD], dt.float16, tag=f"vp{t}", name=f"vp{t}")
            for t in range(T // 128)
        ]
        outTall_sb = singles.tile([128, KC, TS], dt.float16)

        # A2A pieces: P0 after qc3 (512 tok/dest), P1 after qc5 (256),
        # P2 after qc7 (256). dim0 = dest core for in, src core for out.
        a2a_in = [
            dram.tile([N_CORES, HD, n], dt.float16, tag=f"a2a_in{p}", bufs=1,
                      name=f"a2a_in{p}")
            for p, n in ((0, 512), (1, 256), (2, 128), (3, 128))
        ]
        a2a_out = [
            dram.tile([N_CORES, HD, n], dt.float16, tag=f"a2a_out{p}", bufs=1,
                      name=f"a2a_out{p}")
            for p, n in ((0, 512), (1, 256), (2, 128), (3, 128))
        ]
        # outTall col ranges fed by each piece
        piece_cols = [(0, 512), (512, 768), (768, 896), (896, 1024)]

        # act table warm-up: dummy activations with no data deps load the
        # ACT table set (~2.7us) during the DMA window instead of at the
        # first real exp. Warming BOTH Exp and Ln makes the table pass
        # resolve to natural_log_exp_and_others (which contains both) so
        # the tail-qc Ln never triggers a mid-kernel set switch.
        warm_in = singles.tile([1, 16], dt.float32)
        warm_out = singles.tile([1, 16], dt.float16)
        nc.vector.memset(warm_in[:], 1.0)
        nc.scalar.activation(
            warm_out[:], warm_in[:], mybir.ActivationFunctionType.Ln
        )
        nc.scalar.activation(
            warm_out[:], warm_in[:], mybir.ActivationFunctionType.Exp
        )

        # ---------- weights + constants ----------
        # qk weights first (they gate the first production chains), then
        # the first two xT blocks, then the rest of the weights.
        def w_dma(w_sb, w_ext, n):
            e = w_ext[:]
            nc.sync.dma_start(
                out=w_sb[:],
                in_=bass.AP(
                    tensor=e.tensor, offset=e.offset,
                    ap=[list(e.ap[0]), [n, KC], [1, n]],
                ),
            )

        w_dma(wkT_sb, wkT_ext, HD)
        w_dma(wqT_sb, wqT_ext, HD)
        nc.vector.memset(ones_sb[:], 1.0)
        nc.vector.memset(ones65h[:], 1.0)
        for t in range(T // 128):
            nc.vector.memset(vp_t[t][:, 0:1], 1.0)

        # ---------- xT DMA, token-major ----------
        # One DMA per (blk, k): a contiguous 256KB read from the
        # host-packed xTb (vs ~30GB/s/queue for the old 1KB-segmented
        # slices of [D, T]). DMA issue instructions still cost ~0.62us
        # each on the Sync queue, so 4 issues per block is the sweet
        # spot. All on Sync: GpSimd DMA is SWDGE with worse latency.
        def xt_blk_dma(blk):
            for k in range(KC):
                t_ap = xT_k[k][:]
                r0 = (blk * KC + k) * 128
                e_ap = xTb_ext[r0:r0 + 128, :]
                nc.sync.dma_start(
                    out=bass.AP(
                        tensor=t_ap.tensor, offset=t_ap.offset + blk * 512,
                        ap=[list(t_ap.ap[0]), [N, B], [1, 512]],
                    ),
                    in_=e_ap,
                )

        xt_blk_dma(0)
        xt_blk_dma(1)
        w_dma(wvT_sb, wvT_ext, HD)
        for blk in range(2, NQC):
            xt_blk_dma(blk)
        # proj weight is not needed until ~150us in; issue last
        w_dma(wpT_sb, wpT_ext, D)

        # ---------- attention ----------
        with (
            tc.tile_pool(name="pst", bufs=2, space="PSUM") as pst,
            tc.tile_pool(name="pacc", bufs=4, space="PSUM") as pacc,
        ):
            def produce_kq(w_sb, dst, blk, pname):
                ps = pacc.tile([128, 512], dt.float32, tag="acc", name=f"{pname}{blk}")
                for k in range(KC):
                    nc.tensor.matmul(
                        ps[0:64, :],
                        lhsT=w_sb[:, k, :],
                        rhs=xT_k[k][:, blk * 512:(blk + 1) * 512],
                        start=(k == 0), stop=(k == KC - 1),
                        tile_position=(0, 0),
                    )
                    nc.tensor.matmul(
                        ps[64:128, :],
                        lhsT=w_sb[:, k, :],
                        rhs=xT_k[k][:, N + blk * 512:N + (blk + 1) * 512],
                        start=(k == 0), stop=(k == KC - 1),
                        tile_position=(0, 64),
                    )
                nc.vector.tensor_copy(dst[:], ps[:])

            def produce_v(t):
                pv = pacc.tile([128, HD], dt.float32, tag="acc", name=f"pv{t}")
                for k in range(KC):
                    nc.tensor.matmul(
                        pv[:],
                        lhsT=xT_k[k][:, t * 128:(t + 1) * 128],
                        rhs=wvT_sb[:, k, :],
                        start=(k == 0), stop=(k == KC - 1),
                    )
                nc.vector.tensor_copy(vp_t[t][:, 1:1 + HD], pv[:])

            def emit_scores_exp(qc, kt):
                st = pst.tile([128, B, QC], dt.float32, tag="st",
                              name=f"st{qc}_{kt}")
                for pair in range(B):
                    pb = pair * 64
                    lhs_k = kT_t[kt // 4][pb:pb + 64,
                                          (kt % 4) * 128:(kt % 4) * 128 + 128]
                    nc.tensor.matmul(
                        st[:, pair, :],
                        lhsT=lhs_k,
                        rhs=qT_t[qc][pb:pb + 64, :],
                        start=True,
                        stop=True,
                        tile_position=(pb, 0),
                    )
                e = cpool.tile([128, B, QC], dt.float16, tag="e", bufs=32,
                               name=f"e{qc}_{kt}")
                nc.scalar.activation(
                    e[:], st[:], mybir.ActivationFunctionType.Exp, scale=SCALE
                )
                return e

            # normalization: reciprocal of the denominator row, partition
            # broadcast, fused scale-multiply, A2A slice scatter. Pairs are
            # interleaved to halve the serial latency. qc0-6 broadcast via
            # a DRAM bounce (off every engine); qc7 — the exposed tail —
            # broadcasts via a K=1 PE matmul instead (no DMA hops; the
            # PSUM ring is free by then).
            def emit_norm(qc, accs):
                # Evacuate the accumulators to SBUF FIRST: the acc PSUM ring
                # slot then recycles after one DVE copy (~0.9us) instead of
                # after the whole recip->bounce->mul chain (~5-6us, worse
                # under collective DMA traffic) — this was blocking the next
                # qc's first PV, and behind it the scores/exp stream.
                accSs = []
                for pair in range(B):
                    accS = cpool.tile([1 + HD, QC], dt.float32, tag="accS",
                                      name=f"aS{qc}_{pair}")
                    if qc == NQC - 1:
                        # pre-scale by 1/256 so the ScalarE exp(-ln(den'))
                        # below lands in fp16 normal range: den/256 in
                        # [~8, ~120] => recip' = 256/den in [2e-3, 0.13];
                        # the 256s cancel in outTn = accS' * bc'
                        nc.vector.tensor_scalar_mul(
                            accS[:], accs[pair][:], 1.0 / 256.0
                        )
                    else:
                        nc.vector.tensor_copy(accS[:], accs[pair][:])
                    accSs.append(accS)
                bcs = []
                if qc < NQC - 1:
                    rvs = []
                    for pair in range(B):
                        rvec = cpool.tile([1, QC], dt.float32, tag="rvec",
                                          name=f"rv{qc}_{pair}")
                        nc.vector.reciprocal(rvec[:], accSs[pair][0:1, :])
                        rvs.append(rvec)
                    rds = []
                    for pair in range(B):
                        rdram = dram.tile([1, QC], dt.float32, tag="rdram")
                        nc.sync.dma_start(out=rdram[:], in_=rvs[pair][:])
                        rds.append(rdram)
                    for pair in range(B):
                        bcast = cpool.tile([1 + HD, QC], dt.float32,
                                           tag="bcast")
                        r_ap = rds[pair][:]
                        nc.sync.dma_start(
                            out=bcast[:],
                            in_=bass.AP(
                                tensor=r_ap.tensor, offset=r_ap.offset,
                                ap=[[0, 1 + HD]] + list(r_ap.ap[1:]),
                            ),
                        )
                        bcs.append((bcast, accSs[pair]))
                else:
                    # tail qc: the DVE iterative-divide reciprocal costs
                    # 3.3us per pair, serialized, right on the exposed
                    # tail. ScalarE is idle once the last exp retires, and
                    # Ln+Exp live in one table set — recip via
                    # exp(-ln(den')) costs 2x0.72us there instead. Then a
                    # K=1 fp16 PE matmul broadcasts it (an fp32 matmul
                    # here runs LOW_HIGH dual-pass ~2.2us; fp16 ~0.5us).
                    for pair in range(B):
                        lnv = cpool.tile([1, QC], dt.float32, tag="rvec",
                                         name=f"ln{qc}_{pair}")
                        # read the denominator straight from PSUM (not the
                        # evacuated copy) so the Ln starts ~0.9us earlier;
                        # ACT's free affine applies the 1/256 there
                        nc.scalar.activation(
                            lnv[:], accs[pair][0:1, :],
                            mybir.ActivationFunctionType.Ln,
                            scale=1.0 / 256.0,
                        )
                        rvh = cpool.tile([1, QC], dt.float16, tag="rvec",
                                         name=f"rvh{qc}_{pair}")
                        nc.scalar.activation(
                            rvh[:], lnv[:],
                            mybir.ActivationFunctionType.Exp, scale=-1.0,
                        )
                        bc = pacc.tile([1 + HD, QC], dt.float32, tag="acc",
                                       name=f"bc{qc}_{pair}")
                        nc.tensor.matmul(
                            bc[:], lhsT=ones65h[:], rhs=rvh[:],
                            start=True, stop=True,
                        )
                        bcs.append((bc, accSs[pair]))
                for pair in range(B):
                    src_a, src_b = bcs[pair]
                    outTn = cpool.tile([1 + HD, QC], dt.float16, tag="outTn",
                                       name=f"oTn{qc}_{pair}")
                    nc.vector.tensor_mul(outTn[:], src_b[:], src_a[:])
                    # scatter into the A2A piece buffers — one multi-dim
                    # DMA per pair (descriptor issue costs ~0.6us on the
                    # queue; 4 separate issues serialized ~2.4us of the
                    # qc7->trigger critical path)
                    o_ap = outTn[1:1 + HD, 0:QC]
                    if qc < 4:
                        # dst j = qc*4 + pair*2 + h, h=0..1: adjacent j,
                        # same pos half; [64, 2, 256] on both sides
                        j0 = qc * 4 + pair * 2
                        dest0, pos = j0 % N_CORES, j0 // N_CORES
                        d_ap = a2a_in[0][:]
                        nc.sync.dma_start(
                            out=bass.AP(
                                tensor=d_ap.tensor,
                                offset=d_ap.offset + dest0 * (HD * 512)
                                + pos * 256,
                                ap=[[512, HD], [HD * 512, 2], [1, 256]],
                            ),
                            in_=bass.AP(
                                tensor=o_ap.tensor, offset=o_ap.offset,
                                ap=[list(o_ap.ap[0]), [256, 2], [1, 256]],
                            ),
                        )
                    elif qc < 6:
                        j0 = (qc - 4) * 4 + pair * 2
                        d_ap = a2a_in[1][:]
                        nc.sync.dma_start(
                            out=bass.AP(
                                tensor=d_ap.tensor,
                                offset=d_ap.offset + j0 * (HD * 256),
                                ap=[[256, HD], [HD * 256, 2], [1, 256]],
                            ),
                            in_=bass.AP(
                                tensor=o_ap.tensor, offset=o_ap.offset,
                                ap=[list(o_ap.ap[0]), [256, 2], [1, 256]],
                            ),
                        )
                    else:
                        # both pairs on Sync: HWDGE issue+transfer beats
                        # the GpSimd SWDGE path by ~1.4us, and with the
                        # merged single-descriptor writes the Sync queue
                        # has room for both before the trigger
                        piece = 2 if qc == 6 else 3
                        eng = nc.sync
                        j0 = pair * 4
                        d_ap = a2a_in[piece][:]
                        eng.dma_start(
                            out=bass.AP(
                                tensor=d_ap.tensor,
                                offset=d_ap.offset + j0 * (HD * 128),
                                ap=[[128, HD], [HD * 128, 4], [1, 128]],
                            ),
                            in_=bass.AP(
                                tensor=o_ap.tensor, offset=o_ap.offset,
                                ap=[list(o_ap.ap[0]), [128, 4], [1, 128]],
                            ),
                        )
                if qc == 3:
                    trigger_piece(0)
                elif qc == 5:
                    trigger_piece(1)
                elif qc == 6:
                    trigger_piece(2)
                elif qc == 7:
                    trigger_piece(3)

            def trigger_piece(p):
                nc.gpsimd.collective_compute(
                    "AllToAll",
                    mybir.AluOpType.bypass,
                    replica_groups=[list(range(N_CORES))],
                    ins=[a2a_in[p].opt()],
                    outs=[a2a_out[p].opt()],
                )

            def unpack_piece(p, eng=None):
                # mid-span unpacks ride GpSimd (their collective wait must
                # not block the Sync queue); the final piece uses Sync
                # (nothing left behind it, and HWDGE beats SWDGE). One
                # 3D-AP DMA covers all KC chunks: the [8, HD, n] source is
                # row-uniform, so (core-pair, hd) flattens to the partition
                # dim with a k-stride of 128*n.
                eng = eng or nc.gpsimd
                lo, hi = piece_cols[p]
                n = hi - lo
                s_ap = a2a_out[p][:]
                # split=2 for the exposed final piece: the k01 half lands
                # ~0.8us before the whole transfer would, so proj7's first
                # accumulating matmuls start earlier
                nsplit = 2 if p == 3 else 1
                kc_h = KC // nsplit
                for h in range(nsplit):
                    eng.dma_start(
                        out=outTall_sb[:, h * kc_h:(h + 1) * kc_h, lo:hi],
                        in_=bass.AP(
                            tensor=s_ap.tensor,
                            offset=s_ap.offset + h * kc_h * 128 * n,
                            ap=[[n, 128], [128 * n, kc_h], [1, n]],
                        ),
                    )

            def proj_subtile(ts_i):
                # bias is added on the host (a trivial elementwise pass over
                # the returned output) — saves a K=1 PE matmul per subtile
                # in the PE-saturated qc7/tail region
                yp = pacc.tile([128, D], dt.float32, tag="acc", name=f"yp{ts_i}")
                for k in range(KC):
                    nc.tensor.matmul(
                        yp[:],
                        lhsT=outTall_sb[:, k, ts_i * 128:(ts_i + 1) * 128],
                        rhs=wpT_sb[:, k, :],
                        start=(k == 0),
                        stop=(k == KC - 1),
                    )
                y_sb = cpool.tile([128, D], dt.float16, tag="y", name=f"y{ts_i}")
                nc.vector.tensor_copy(y_sb[:], yp[:])
                # split across two queues so the final output lands faster
                for ph in range(2):
                    p0 = ph * 64
                    nc.sync.dma_start(
                        out=out_ext[ts_i * 128 + p0:ts_i * 128 + p0 + 64, :],
                        in_=y_sb[p0:p0 + 64, :],
                    )

            # pre-loop production (gated on block-0 DMA + weights);
            # kT1 moves to extras[(0,1)] so scores(0,0) isn't queued
            # behind it (walrus reorders around blocked instructions,
            # but emitting it later keeps the first-exp chain tight)
            produce_kq(wkT_sb, kT_t[0], 0, "k")
            produce_kq(wqT_sb, qT_t[0], 0, "q")

            # JIT production schedule: extras[(qc, kt)] emitted right after
            # that slot's exp.
            extras = {}
            extras.setdefault((0, 1), []).append(
                lambda: produce_kq(wkT_sb, kT_t[1], 1, "k"))
            # kT JIT production 2 slots later than before: the exp stream
            # now starts ~10us earlier, so blk2-7's DMA landings need the
            # extra margin (kT_t[blk] is first read at kt=4*blk)
            for blk in range(2, NQC):
                extras.setdefault((0, 4 * blk - 4), []).append(
                    (lambda b: lambda: produce_kq(wkT_sb, kT_t[b], b, "k"))(blk))
            extras.setdefault((0, 26), []).append(
                lambda: produce_kq(wqT_sb, qT_t[1], 1, "q"))
            for i, n_ in enumerate(range(2, 6)):
                extras.setdefault((1, 2 + 8 * i), []).append(
                    (lambda m: lambda: produce_kq(wqT_sb, qT_t[m], m, "q"))(n_))
            for i, n_ in enumerate(range(6, NQC)):
                extras.setdefault((2, 2 + 8 * i), []).append(
                    (lambda m: lambda: produce_kq(wqT_sb, qT_t[m], m, "q"))(n_))

            # PV stream state
            pending = deque()   # (qc, kt, e_tile)
            acc_of = {}         # qc -> [acc_b0, acc_b1]
            v_done = set()
            pv_emitted = 0

            def emit_pv_pair():
                qc, kt, e = pending.popleft()
                if qc not in acc_of:
                    acc_of[qc] = [
                        pacc.tile([1 + HD, QC], dt.float32, tag="acc",
                                  name=f"acc{qc}_{p}")
                        for p in range(B)
                    ]
                for pair in range(B):
                    vidx = pair * NKT + kt
                    if vidx not in v_done:
                        produce_v(vidx)
                        v_done.add(vidx)
                    nc.tensor.matmul(
                        acc_of[qc][pair][:, :],
                        lhsT=vp_t[vidx][:],
                        rhs=e[:, pair, :],
                        start=(kt == 0),
                        stop=(kt == NKT - 1),
                    )
                if kt == NKT - 1:
                    emit_norm(qc, acc_of.pop(qc))

            for qc in range(NQC):
                base = pv_emitted
                for kt in range(NKT):
                    e = emit_scores_exp(qc, kt)
                    pending.append((qc, kt, e))
                    for fn in extras.get((qc, kt), ()):
                        fn()
                    # pace PV emission: quota spread evenly across the qc
                    goal = base + (PV_QUOTA[qc] * (kt + 1)) // NKT
                    while pv_emitted < goal and pending:
                        emit_pv_pair()
                        pv_emitted += 1
                    # late-attention overlap of unpack + proj for the
                    # already-landed pieces. qc6 has PE slack (no
                    # production, no drain pressure) while qc7's slots are
                    # saturated — so subtiles 0-3 (piece-0 data, landed
                    # ~40us earlier) run in qc6, and only 4-5 (piece-1
                    # data) in early qc7, leaving the last 20 exp slots
                    # free of PE competition.
                    if qc == 6:
                        if kt == 2:
                            unpack_piece(0)
                        if kt in (6, 12, 18, 24):
                            proj_subtile((kt - 6) // 6)
                    elif qc == 7:
                        if kt in (6, 12):
                            proj_subtile(4 + (kt - 6) // 6)
                # P1 is complete well before qc6 ends
                if qc == 6:
                    unpack_piece(1)

            while pending:
                emit_pv_pair()
                pv_emitted += 1

            # keep the PE HAM-warm through the last A2A flight so the final
            # proj subtile runs at full clock
            # piece 2's unpack + proj hide entirely under piece 3's A2A
            # flight; keep-warm dummies fill the remaining PE idle so the
            # final proj subtile runs at full clock
            unpack_piece(2)
            proj_subtile(6)
            scratch = pacc.tile([1, D], dt.float32, tag="acc", name="scratch")
            for _ in range(24):
                nc.tensor.matmul(
                    scratch[:, 0:128], lhsT=ones_sb[:, 0:1], rhs=ones_sb[:],
                    start=True, stop=True,
                )
            unpack_piece(3, eng=nc.sync)
            proj_subtile(7)

    return nc


def _get_nc():
    if "nc" not in _COMPILED:
        _COMPILED["nc"] = _build()
    return _COMPILED["nc"]


def _seg_token(piece, j):
    """Map (piece, subchunk index) -> (batch, token start, length)."""
    if piece == 0:
        qc = j // 4
        rem = j % 4
        return rem // 2, qc * 512 + (rem % 2) * 256, 256
    if piece == 1:
        qc = 4 + j // 4
        rem = j % 4
        return rem // 2, qc * 512 + (rem % 2) * 256, 256
    qc = 6 if piece == 2 else 7
    return j // 4, qc * 512 + (j % 4) * 128, 128


def kernel(x, w_qkv, w_proj, b_proj):
    from concourse.bass_utils import run_bass_kernel_spmd

    x = np.asarray(x, dtype=np.float32)
    w_qkv = np.asarray(w_qkv, dtype=np.float32)
    w_proj = np.asarray(w_proj, dtype=np.float32)
    b_proj = np.asarray(b_proj, dtype=np.float32)

    # Pack every tensor so the device DMAs are contiguous reads (see the
    # declare_dram_parameter comment in _build).
    # xTb[(blk*KC+k)*128+p, b*512+c] = x[b, blk*512+c, k*128+p]
    xTb = np.ascontiguousarray(
        x.transpose(2, 0, 1)                   # [D, B, N]
        .reshape(KC, 128, B, NQC, 512)
        .transpose(3, 0, 1, 2, 4)              # [NQC, KC, 128, B, 512]
        .reshape(NQC * KC * 128, B * 512)
    ).astype(F16)

    def pack_w(wT, n):
        # wT [D, n] -> [128, KC*n] with (p, k*n+c) = wT[k*128+p, c]
        return np.ascontiguousarray(
            wT.reshape(KC, 128, n).transpose(1, 0, 2).reshape(128, KC * n)
        ).astype(F16)

    wpp = pack_w(w_proj.T, D)

    in_maps = []
    for d in range(N_CORES):
        wq = w_qkv[0 * D + d * HD: 0 * D + (d + 1) * HD, :]
        wk = w_qkv[1 * D + d * HD: 1 * D + (d + 1) * HD, :]
        wv = w_qkv[2 * D + d * HD: 2 * D + (d + 1) * HD, :]
        in_maps.append({
            "xTb": xTb,
            "wqp": pack_w(wq.T, HD),
            "wkp": pack_w(wk.T, HD),
            "wvp": pack_w(wv.T, HD),
            "wpp": wpp,
        })

    nc = _get_nc()
    res = run_bass_kernel_spmd(nc, in_maps, core_ids=list(range(N_CORES)))

    y = np.empty((B, N, D), dtype=np.float32)
    for s in range(N_CORES):
        r = np.asarray(res.results[s]["out"], dtype=np.float32)
        segs = [(0, s), (0, s + 8), (1, s), (2, s), (3, s)]
        row = 0
        for piece, j in segs:
            b, t0, ln = _seg_token(piece, j)
            y[b, t0:t0 + ln, :] = r[row:row + ln, :]
            row += ln
    # proj bias is applied here (fp32) rather than on-device: it is a
    # trivial elementwise pass and its K=1 matmuls were stealing PE slots
    # in the saturated qc7/tail region
    y += b_proj
    return y

